# revision 1
# baseline (speedup 1.0000x reference)
"""Llama4-style MoE (8 experts, top-1, + shared SwiGLU MLP) on 8 Trainium2 cores.

Strategy (expert-parallel + sparse top-1):
  - every core receives the full hidden_states (x and x^T), its own expert's
    gate_up/down weights, a 1/8 slice of the shared MLP (tensor-parallel over
    the intermediate dim), and router weights rolled so that "its" expert is
    column 0.
  - on device: fp32 router matmul -> top-1 mask + sigmoid score -> prefix-sum
    compaction (selection-matrix matmuls) -> bf16 expert MLP on <=C packed
    tokens -> indirect-DMA scatter-add into a [T,H] partial that also holds
    the shared-MLP partial -> ReduceScatter over the 8 cores.
  - host: concatenates the 8 [T/8, H] shards.
"""
import sys

if '/opt/trn_rl_repo' not in sys.path:
    sys.path.insert(0, '/opt/trn_rl_repo')

import numpy as np

import concourse.bass as bass
import concourse.bacc as bacc
import concourse.mybir as mybir
import concourse.tile as tile
from concourse.bass_utils import run_bass_kernel_spmd

dt = mybir.dt
AF = mybir.ActivationFunctionType
OP = mybir.AluOpType
P = 128


class Cfg:
    def __init__(self, n_cores=8, T=2048, H=2048, I=4096, C=384):
        self.n_cores, self.T, self.H, self.I, self.C = n_cores, T, H, I, C
        self.E = 8
        self.IS = I // n_cores        # shared-MLP intermediate slice per core
        self.TSH = T // n_cores       # output shard rows per core
        self.HK = H // P              # contraction chunks over H
        self.TJ = T // P              # token chunks
        self.NI = I // P              # I tiles
        self.CT = C // P              # packed-slot tiles
        assert C % P == 0 and T % P == 0 and H % P == 0 and I % P == 0
        assert self.IS % P == 0 and self.TJ % 2 == 0


def _nmax(v, cap=512):
    out = []
    o = 0
    while o < v:
        s = min(cap, v - o)
        out.append((o, s))
        o += s
    return out


def build(cfg: Cfg, taps: bool = False):
    T, H, I, C = cfg.T, cfg.H, cfg.I, cfg.C
    HK, TJ, NI, CT, IS = cfg.HK, cfg.TJ, cfg.NI, cfg.CT, cfg.IS
    ISK = IS // P
    TH = T // 2                    # half of the tokens (x^T retained by half)
    TJH = TJ // 2
    BIGC = 1.0e5

    nc = bacc.Bacc("TRN2", target_bir_lowering=False, debug=False,
                   num_devices=cfg.n_cores)

    tap_d = {}
    if taps:
        for name, shape, dty in [
                ("t_logits", [P, TJ * 8], dt.float32),
                ("t_posm", [P, TJ], dt.float32),
                ("t_dest", [P, CT], dt.int32),
                ("t_xhat", [P, HK * C], dt.float32),
                ("t_routed", [P, CT * H], dt.float32),
                ("t_part", [T, H], dt.float32)]:
            tap_d[name] = nc.dram_tensor(name, shape, dty,
                                         kind="ExternalOutput").ap()

    xT_d = nc.dram_tensor("xT", [H, T], dt.float32, kind="ExternalInput").ap()
    x_d = nc.dram_tensor("x", [T, H], dt.float32, kind="ExternalInput").ap()
    rwT_d = nc.dram_tensor("rwT", [H, 8], dt.float32, kind="ExternalInput").ap()
    wgu_d = nc.dram_tensor("wgu", [H, 2 * I], dt.float32,
                           kind="ExternalInput").ap()
    wd_d = nc.dram_tensor("wd", [I, H], dt.float32, kind="ExternalInput").ap()
    wgs_d = nc.dram_tensor("wgs", [H, IS], dt.float32,
                           kind="ExternalInput").ap()
    wus_d = nc.dram_tensor("wus", [H, IS], dt.float32,
                           kind="ExternalInput").ap()
    wds_d = nc.dram_tensor("wds", [IS, H], dt.float32,
                           kind="ExternalInput").ap()
    y_d = nc.dram_tensor("y", [cfg.TSH, H], dt.float32,
                         kind="ExternalOutput").ap()

    with tile.TileContext(nc) as tc:
        with tc.tile_pool(name="const", bufs=1) as const, \
             tc.tile_pool(name="keep", bufs=1) as keep, \
             tc.tile_pool(name="sbuf", bufs=3) as sb, \
             tc.tile_pool(name="stream", bufs=3) as stream, \
             tc.tile_pool(name="pps", bufs=2, space="PSUM") as pps, \
             tc.tile_pool(name="pbig", bufs=4, space="PSUM") as pbig, \
             tc.tile_pool(name="dram", bufs=1, space="DRAM") as dram:

            part = dram.tile([T, H], dt.float32)
            rs_out = dram.tile([cfg.TSH, H], dt.float32)

            # ---------------- constants ----------------
            iota_col_i = const.tile([P, P], dt.int32)
            nc.gpsimd.iota(iota_col_i[:], pattern=[[1, P]], base=0,
                           channel_multiplier=0)
            iota_row_i = const.tile([P, P], dt.int32)
            nc.gpsimd.iota(iota_row_i[:], pattern=[[0, P]], base=0,
                           channel_multiplier=1)
            iota_col_f = const.tile([P, P], dt.float32)
            nc.vector.tensor_copy(iota_col_f[:], iota_col_i[:])
            iota_row_f = const.tile([P, P], dt.float32)
            nc.vector.tensor_copy(iota_row_f[:], iota_row_i[:])
            ltri = const.tile([P, P], dt.bfloat16)  # ltri[k,m] = 1 if k<m
            nc.vector.tensor_tensor(out=ltri[:], in0=iota_row_f[:],
                                    in1=iota_col_f[:], op=OP.is_lt)

            iotaC_i = const.tile([P, C], dt.int32)
            nc.gpsimd.iota(iotaC_i[:], pattern=[[1, C]], base=0,
                           channel_multiplier=0)
            iotaC_f = const.tile([P, C], dt.float32)
            nc.vector.tensor_copy(iotaC_f[:], iotaC_i[:])

            # empty packed slots get an out-of-bounds row (>= T); the scatter
            # uses bounds_check with oob_is_err=False so they are skipped.
            trash_i = const.tile([P, CT], dt.int32)
            nc.gpsimd.iota(trash_i[:], pattern=[[P, CT]], base=T,
                           channel_multiplier=1)
            trash_f = const.tile([P, CT], dt.float32)
            nc.vector.tensor_copy(trash_f[:], trash_i[:])

            # rhs for the dest matmul, all bf16-exact (<=128):
            # col0 = p (token lo), col1 = tj (token hi), col2 = 1
            lo_i = const.tile([P, TJ], dt.int32)
            nc.gpsimd.iota(lo_i[:], pattern=[[0, TJ]], base=0,
                           channel_multiplier=1)
            hi_i = const.tile([P, TJ], dt.int32)
            nc.gpsimd.iota(hi_i[:], pattern=[[1, TJ]], base=0,
                           channel_multiplier=0)
            tokone = const.tile([P, TJ, 3], dt.bfloat16)
            nc.vector.tensor_copy(tokone[:, :, 0], lo_i[:])
            nc.vector.tensor_copy(tokone[:, :, 1], hi_i[:])
            nc.vector.memset(tokone[:, :, 2], 1.0)

            ones_col_bf = const.tile([P, 1], dt.bfloat16)
            nc.vector.memset(ones_col_bf[:], 1.0)
            ones_row_bf = const.tile([1, P], dt.bfloat16)
            nc.vector.memset(ones_row_bf[:], 1.0)

            # rolled router weights [P, HK, 8] fp32
            rw_sb = const.tile([P, HK, 8], dt.float32)
            nc.sync.dma_start(rw_sb[:],
                              rwT_d.rearrange("(hk p) e -> p hk e", p=P))

            # expert activations (P8->P9)
            act_cm = tc.tile_pool(name="apool", bufs=1)
            apool = act_cm.__enter__()
            # mid-lived: selection matrices + packed activations
            mid_cm = tc.tile_pool(name="mid", bufs=1)
            mid = mid_cm.__enter__()

            logits = keep.tile([P, TJ, 8], dt.float32)
            act_sT = keep.tile([P, ISK, T], dt.bfloat16)

            # ==== P1+P5 (by token half): fp32 router; shared-MLP gate/up ====
            with tc.tile_pool(name="xtbf", bufs=1) as xtbf_pool, \
                 tc.tile_pool(name="ppr", bufs=2, space="PSUM") as ppr:
                for th in range(2):
                    xtbf = xtbf_pool.tile([P, HK, TH], dt.bfloat16,
                                          tag="xtbf")
                    for tjl in range(TJH):
                        tj = th * TJH + tjl
                        xcol = stream.tile([P, HK, P], dt.float32,
                                           tag="stg_f")
                        nc.sync.dma_start(
                            xcol[:], xT_d[:, tj * P:(tj + 1) * P]
                            .rearrange("(hk p) t -> p hk t", p=P))
                        nc.scalar.activation(
                            xtbf[:, :, tjl * P:(tjl + 1) * P], xcol[:],
                            AF.Copy)
                        pl = ppr.tile([P, 8], dt.float32, tag="plog")
                        for hk in range(HK):
                            nc.tensor.matmul(
                                pl[:], xcol[:, hk, :], rw_sb[:, hk, :],
                                start=(hk == 0), stop=(hk == HK - 1))
                        nc.vector.tensor_copy(logits[:, tj, :], pl[:])

                    # shared gate/up on this token half
                    for isx in range(ISK):
                        wg_f = stream.tile([P, HK, P], dt.float32,
                                           tag="stg_f")
                        nc.sync.dma_start(
                            wg_f[:], wgs_d[:, isx * P:(isx + 1) * P]
                            .rearrange("(hk p) c -> p hk c", p=P))
                        wg_b = stream.tile([P, HK, P], dt.bfloat16,
                                           tag="stg_b")
                        nc.vector.tensor_copy(wg_b[:], wg_f[:])
                        wu_f = stream.tile([P, HK, P], dt.float32,
                                           tag="stg_f")
                        nc.sync.dma_start(
                            wu_f[:], wus_d[:, isx * P:(isx + 1) * P]
                            .rearrange("(hk p) c -> p hk c", p=P))
                        wu_b = stream.tile([P, HK, P], dt.bfloat16,
                                           tag="stg_b")
                        nc.scalar.activation(wu_b[:], wu_f[:], AF.Copy)
                        for tn, tw in _nmax(TH):
                            pg = pbig.tile([P, 512], dt.float32, tag="pbig")
                            pu = pbig.tile([P, 512], dt.float32, tag="pbig")
                            for hk in range(HK):
                                nc.tensor.matmul(pg[:, :tw], wg_b[:, hk, :],
                                                 xtbf[:, hk, tn:tn + tw],
                                                 start=(hk == 0),
                                                 stop=(hk == HK - 1))
                            for hk in range(HK):
                                nc.tensor.matmul(pu[:, :tw], wu_b[:, hk, :],
                                                 xtbf[:, hk, tn:tn + tw],
                                                 start=(hk == 0),
                                                 stop=(hk == HK - 1))
                            sil = sb.tile([P, 512], dt.float32, tag="sil")
                            nc.scalar.activation(sil[:, :tw], pg[:, :tw],
                                                 AF.Silu)
                            nc.vector.tensor_tensor(
                                out=act_sT[:, isx,
                                           th * TH + tn:th * TH + tn + tw],
                                in0=sil[:, :tw], in1=pu[:, :tw], op=OP.mult)

            if taps:
                nc.sync.dma_start(
                    tap_d["t_logits"][:],
                    logits[:].rearrange("p tj e -> p (tj e)"))

            # ============ P2: top-1 mask + sigmoid score ============
            maxv = keep.tile([P, TJ], dt.float32)
            for tj in range(TJ):
                m8 = sb.tile([P, 8], dt.float32, tag="m8")
                nc.vector.max(m8[:], logits[:, tj, :])
                nc.vector.tensor_copy(maxv[:, tj:tj + 1], m8[:, 0:1])
            sig = keep.tile([P, TJ], dt.float32)
            nc.scalar.activation(sig[:], maxv[:], AF.Sigmoid)
            mask = keep.tile([P, TJ], dt.float32)
            nc.vector.tensor_tensor(out=mask[:], in0=logits[:, :, 0],
                                    in1=maxv[:], op=OP.is_equal)
            smine = keep.tile([P, TJ], dt.float32)
            nc.vector.tensor_tensor(out=smine[:], in0=mask[:], in1=sig[:],
                                    op=OP.mult)
            mask_bf = keep.tile([P, TJ], dt.bfloat16)
            nc.vector.tensor_copy(mask_bf[:], mask[:])

            # ============ P3: packed positions (prefix sums) ============
            pos_ps = pps.tile([P, TJ], dt.float32, bufs=1, tag="pos")
            nc.tensor.matmul(pos_ps[:], ltri[:], mask_bf[:],
                             start=True, stop=True)
            tot_ps = pps.tile([1, TJ], dt.float32, bufs=1, tag="tb")
            nc.tensor.matmul(tot_ps[:], ones_col_bf[:], mask_bf[:],
                             start=True, stop=True)
            tot_bf = sb.tile([1, TJ], dt.bfloat16)
            nc.vector.tensor_copy(tot_bf[:], tot_ps[:])
            bc_ps = pps.tile([P, TJ], dt.float32, bufs=1, tag="tb")
            nc.tensor.matmul(bc_ps[:], ones_row_bf[:], tot_bf[:],
                             start=True, stop=True)
            # exclusive scan along the TJ axis of the broadcast totals
            exa = sb.tile([P, TJ], dt.float32, tag="scan")
            nc.vector.memset(exa[:, 0:1], 0.0)
            if TJ > 1:
                nc.vector.tensor_copy(exa[:, 1:], bc_ps[:, :TJ - 1])
            sh = 1
            while sh < TJ:
                exb = sb.tile([P, TJ], dt.float32, tag="scan")
                nc.vector.tensor_copy(exb[:, :sh], exa[:, :sh])
                nc.vector.tensor_tensor(out=exb[:, sh:], in0=exa[:, sh:],
                                        in1=exa[:, :TJ - sh], op=OP.add)
                exa = exb
                sh *= 2
            posg = keep.tile([P, TJ], dt.float32)
            nc.vector.tensor_tensor(out=posg[:], in0=exa[:], in1=pos_ps[:],
                                    op=OP.add)
            nmsk = sb.tile([P, TJ], dt.float32, tag="scan")
            nc.vector.tensor_scalar(out=nmsk[:], in0=mask[:],
                                    scalar1=-BIGC, scalar2=BIGC,
                                    op0=OP.mult, op1=OP.add)
            posm = keep.tile([P, TJ], dt.float32)
            nc.vector.tensor_tensor(out=posm[:], in0=posg[:], in1=nmsk[:],
                                    op=OP.add)
            if taps:
                nc.sync.dma_start(tap_d["t_posm"][:], posm[:])

            # ============ P4: selection matrices ============
            S_bf = mid.tile([P, TJ, C], dt.bfloat16)
            S01b = mid.tile([P, TJ, C], dt.bfloat16)
            for tj in range(TJ):
                s01 = sb.tile([P, C], dt.float32, tag="s01")
                nc.vector.tensor_tensor(
                    out=s01[:],
                    in0=posm[:, tj:tj + 1].to_broadcast([P, C]),
                    in1=iotaC_f[:], op=OP.is_equal)
                nc.vector.tensor_copy(S01b[:, tj, :], s01[:])
                nc.vector.tensor_tensor(
                    out=S_bf[:, tj, :], in0=s01[:],
                    in1=smine[:, tj:tj + 1].to_broadcast([P, C]),
                    op=OP.mult)

            # ============ P10: shared down-proj -> part[t, :] ============
            wds_cm = tc.tile_pool(name="wpool", bufs=1)
            wpool = wds_cm.__enter__()
            wds_b = wpool.tile([P, ISK, H], dt.bfloat16)
            for ik in range(ISK):
                wds_f = stream.tile([P, H], dt.float32, tag="stg_f")
                nc.sync.dma_start(wds_f[:], wds_d[ik * P:(ik + 1) * P, :])
                nc.vector.tensor_copy(wds_b[:, ik, :], wds_f[:])
            for tt in range(TJ):
                for hn, hw in _nmax(H):
                    psd = pbig.tile([P, 512], dt.float32, tag="pbig")
                    for ik in range(ISK):
                        nc.tensor.matmul(psd[:, :hw],
                                         act_sT[:, ik, tt * P:(tt + 1) * P],
                                         wds_b[:, ik, hn:hn + hw],
                                         start=(ik == 0),
                                         stop=(ik == ISK - 1))
                    so = sb.tile([P, 512], dt.float32, tag="sil")
                    nc.vector.tensor_copy(so[:, :hw], psd[:, :hw])
                    nc.sync.dma_start(
                        part[tt * P:(tt + 1) * P, hn:hn + hw], so[:, :hw])
            wds_cm.__exit__(None, None, None)

            # ============ P6: token compaction x_hat^T = x^T @ S ============
            xhat = mid.tile([P, HK, C], dt.bfloat16)
            for hm in range(HK):
                xb_f = stream.tile([P, TJ, P], dt.float32, tag="stg_f")
                nc.sync.dma_start(
                    xb_f[:], x_d[:, hm * P:(hm + 1) * P]
                    .rearrange("(tj p) h -> p tj h", p=P))
                xb_b = stream.tile([P, TJ, P], dt.bfloat16, tag="stg_b")
                nc.scalar.activation(xb_b[:], xb_f[:], AF.Copy)
                px = pbig.tile([P, C], dt.float32, tag="pbig")
                for tj in range(TJ):
                    nc.tensor.matmul(px[:], xb_b[:, tj, :], S_bf[:, tj, :],
                                     start=(tj == 0), stop=(tj == TJ - 1))
                nc.vector.tensor_copy(xhat[:, hm, :], px[:])

            # ============ P7: output row index per packed slot ============
            dest_i = keep.tile([P, CT], dt.int32)
            for sc in range(CT):
                pd = pps.tile([P, 3], dt.float32, bufs=1, tag="tb")
                for tj in range(TJ):
                    nc.tensor.matmul(pd[:], S01b[:, tj, sc * P:(sc + 1) * P],
                                     tokone[:, tj, :],
                                     start=(tj == 0), stop=(tj == TJ - 1))
                # dest = lo + 128*hi  if occupied else trash row
                t1 = sb.tile([P, 1], dt.float32, tag="dsmall")
                nc.vector.tensor_scalar(out=t1[:], in0=pd[:, 1:2],
                                        scalar1=float(P), scalar2=None,
                                        op0=OP.mult)
                t1b = sb.tile([P, 1], dt.float32, tag="dsmall")
                nc.vector.tensor_tensor(out=t1b[:], in0=t1[:], in1=pd[:, 0:1],
                                        op=OP.add)
                t2 = sb.tile([P, 1], dt.float32, tag="dsmall")
                nc.vector.tensor_scalar(out=t2[:], in0=pd[:, 2:3],
                                        scalar1=-1.0, scalar2=1.0,
                                        op0=OP.mult, op1=OP.add)
                t3 = sb.tile([P, 1], dt.float32, tag="dsmall")
                nc.vector.tensor_tensor(out=t3[:], in0=t2[:],
                                        in1=trash_f[:, sc:sc + 1], op=OP.mult)
                t4 = sb.tile([P, 1], dt.float32, tag="dsmall")
                nc.vector.tensor_tensor(out=t4[:], in0=t3[:], in1=t1b[:],
                                        op=OP.add)
                nc.vector.tensor_copy(dest_i[:, sc:sc + 1], t4[:])
            if taps:
                nc.sync.dma_start(tap_d["t_dest"][:], dest_i[:])
                for hm in range(HK):
                    xtmp = sb.tile([P, C], dt.float32, tag="s01")
                    nc.vector.tensor_copy(xtmp[:], xhat[:, hm, :])
                    nc.sync.dma_start(
                        tap_d["t_xhat"][:, hm * C:(hm + 1) * C], xtmp[:])

            # ============ P8: expert gate_up^T then act^T ============
            actT = apool.tile([P, NI, C], dt.bfloat16)
            for ii in range(NI):
                wg_f = stream.tile([P, HK, P], dt.float32, tag="stg_f")
                nc.sync.dma_start(
                    wg_f[:], wgu_d[:, ii * P:(ii + 1) * P]
                    .rearrange("(hk p) c -> p hk c", p=P))
                wg_b = stream.tile([P, HK, P], dt.bfloat16, tag="stg_b")
                nc.vector.tensor_copy(wg_b[:], wg_f[:])
                wu_f = stream.tile([P, HK, P], dt.float32, tag="stg_f")
                nc.sync.dma_start(
                    wu_f[:], wgu_d[:, I + ii * P:I + (ii + 1) * P]
                    .rearrange("(hk p) c -> p hk c", p=P))
                wu_b = stream.tile([P, HK, P], dt.bfloat16, tag="stg_b")
                nc.scalar.activation(wu_b[:], wu_f[:], AF.Copy)
                pg = pbig.tile([P, C], dt.float32, tag="pbig")
                pu = pbig.tile([P, C], dt.float32, tag="pbig")
                for hk in range(HK):
                    nc.tensor.matmul(pg[:], wg_b[:, hk, :], xhat[:, hk, :],
                                     start=(hk == 0), stop=(hk == HK - 1))
                for hk in range(HK):
                    nc.tensor.matmul(pu[:], wu_b[:, hk, :], xhat[:, hk, :],
                                     start=(hk == 0), stop=(hk == HK - 1))
                sil = sb.tile([P, C], dt.float32, tag="s01")
                nc.scalar.activation(sil[:], pg[:], AF.Silu)
                nc.vector.tensor_tensor(out=actT[:, ii, :], in0=sil[:],
                                        in1=pu[:], op=OP.mult)

            mid_cm.__exit__(None, None, None)

            # ==== P9: expert down-proj -> packed rows, scatter-add ====
            rt_cm = tc.tile_pool(name="rpool", bufs=1)
            rpool = rt_cm.__enter__()
            routed_sb = rpool.tile([P, CT, H], dt.float32)
            HQ = min(512, H)
            with tc.tile_pool(name="wdh", bufs=1) as wdh_pool:
                for q in range(H // HQ):
                    wdh_b = wdh_pool.tile([P, NI, HQ], dt.bfloat16,
                                          tag="wdh_b")
                    for ik in range(NI):
                        wd_f = stream.tile([P, HQ], dt.float32, tag="stg_f")
                        nc.sync.dma_start(
                            wd_f[:], wd_d[ik * P:(ik + 1) * P,
                                          q * HQ:(q + 1) * HQ])
                        if ik % 2 == 0:
                            nc.vector.tensor_copy(wdh_b[:, ik, :], wd_f[:])
                        else:
                            nc.scalar.activation(wdh_b[:, ik, :], wd_f[:],
                                                 AF.Copy)
                    for ct in range(CT):
                        pdn = pbig.tile([P, HQ], dt.float32, tag="pbig")
                        for ik in range(NI):
                            nc.tensor.matmul(
                                pdn[:], actT[:, ik, ct * P:(ct + 1) * P],
                                wdh_b[:, ik, :],
                                start=(ik == 0), stop=(ik == NI - 1))
                        nc.vector.tensor_copy(
                            routed_sb[:, ct, q * HQ:(q + 1) * HQ], pdn[:])

            if taps:
                nc.sync.dma_start(
                    tap_d["t_routed"][:],
                    routed_sb[:].rearrange("p ct h -> p (ct h)"))

            # scatter-add packed rows into part
            for ct in range(CT):
                nc.gpsimd.indirect_dma_start(
                    out=part[:],
                    out_offset=bass.IndirectOffsetOnAxis(
                        ap=dest_i[:, ct:ct + 1], axis=0),
                    in_=routed_sb[:, ct, :],
                    in_offset=None,
                    bounds_check=T - 1,
                    oob_is_err=False,
                    compute_op=OP.add)
            rt_cm.__exit__(None, None, None)
            act_cm.__exit__(None, None, None)

            if taps:
                nc.sync.dma_start(tap_d["t_part"][:], part[:])

            # ============ P12: ReduceScatter + output ============
            nc.gpsimd.collective_compute(
                "ReduceScatter", OP.add,
                replica_groups=[list(range(cfg.n_cores))],
                ins=[part.opt()],
                outs=[rs_out.opt()])
            for b0, bw in _nmax(cfg.TSH, P):
                ot = stream.tile([P, H], dt.float32, tag="stg_f")
                nc.sync.dma_start(ot[:bw, :], rs_out[b0:b0 + bw, :])
                nc.sync.dma_start(y_d[b0:b0 + bw, :], ot[:bw, :])

    nc.compile()
    return nc


# dims of the real problem
CFG = Cfg(n_cores=8, T=2048, H=2048, I=4096, C=384)
_NC_CACHE = {}


def _get_nc(cfg, taps=False):
    key = (cfg.n_cores, cfg.T, cfg.H, cfg.I, cfg.C, taps)
    if key not in _NC_CACHE:
        _NC_CACHE[key] = build(cfg, taps=taps)
    return _NC_CACHE[key]


def make_in_maps(cfg, hidden_states, router_w, gate_up_proj, down_proj,
                 shared_gate_w, shared_up_w, shared_down_w):
    T, H, IS = cfg.T, cfg.H, cfg.IS
    x = np.ascontiguousarray(
        np.asarray(hidden_states, dtype=np.float32).reshape(T, H))
    xT = np.ascontiguousarray(x.T)
    router_w = np.asarray(router_w, dtype=np.float32)
    in_maps = []
    for c in range(cfg.n_cores):
        rw_roll = np.roll(router_w, -c, axis=0)  # row j = expert (c+j)%8
        in_maps.append({
            "xT": xT,
            "x": x,
            "rwT": np.ascontiguousarray(rw_roll.T),
            "wgu": np.ascontiguousarray(np.asarray(gate_up_proj[c],
                                                   dtype=np.float32)),
            "wd": np.ascontiguousarray(np.asarray(down_proj[c],
                                                  dtype=np.float32)),
            "wgs": np.ascontiguousarray(
                np.asarray(shared_gate_w[:, c * IS:(c + 1) * IS],
                           dtype=np.float32)),
            "wus": np.ascontiguousarray(
                np.asarray(shared_up_w[:, c * IS:(c + 1) * IS],
                           dtype=np.float32)),
            "wds": np.ascontiguousarray(
                np.asarray(shared_down_w[c * IS:(c + 1) * IS, :],
                           dtype=np.float32)),
        })
    return in_maps


def kernel(hidden_states, router_w, gate_up_proj, down_proj,
           shared_gate_w, shared_up_w, shared_down_w):
    cfg = CFG
    orig_shape = np.asarray(hidden_states).shape
    nc = _get_nc(cfg)
    in_maps = make_in_maps(cfg, hidden_states, router_w, gate_up_proj,
                           down_proj, shared_gate_w, shared_up_w,
                           shared_down_w)
    res = run_bass_kernel_spmd(nc, in_maps, core_ids=list(range(cfg.n_cores)))
    y = np.concatenate([res.results[c]["y"] for c in range(cfg.n_cores)],
                       axis=0)
    return y.reshape(orig_shape).astype(np.float32)



# revision 21
# speedup vs baseline: 83.8579x; 83.8579x over previous
"""Llama4-style MoE (8 experts, top-1, + shared SwiGLU MLP) on 8 Trainium2 cores.

Strategy (expert-parallel + sparse top-1):
  - every core receives the full hidden_states (fp32 tiled for the router,
    bf16 row-major for the token gather), its own expert's gate_up/down
    weights and a 1/8 slice of the shared MLP (tensor-parallel over the
    intermediate dim) — all weights pre-cast to bf16 and pre-tiled on the
    host so device DMAs are fully contiguous.
  - on device: fp32 router matmul -> top-1 mask + sigmoid score ->
    prefix-sum compaction indices -> indirect-DMA gather of the <=C routed
    token rows -> score scale -> XBAR DMA transpose to contraction layout
    -> bf16 expert MLP -> indirect-DMA scatter-add into a [T,H] partial
    that also holds the shared-MLP partial -> ReduceScatter over 8 cores.
  - host: concatenates the 8 [T/8, H] shards.
"""
import sys

if '/opt/trn_rl_repo' not in sys.path:
    sys.path.insert(0, '/opt/trn_rl_repo')

import numpy as np
import ml_dtypes

import concourse.bass as bass
import concourse.bacc as bacc
import concourse.mybir as mybir
import concourse.tile as tile
from concourse.bass_utils import run_bass_kernel_spmd

dt = mybir.dt
AF = mybir.ActivationFunctionType
OP = mybir.AluOpType
P = 128
BF16 = ml_dtypes.bfloat16


class Cfg:
    def __init__(self, n_cores=8, T=2048, H=2048, I=4096, C=384):
        self.n_cores, self.T, self.H, self.I, self.C = n_cores, T, H, I, C
        self.E = 8
        self.IS = I // n_cores        # shared-MLP intermediate slice per core
        self.TSH = T // n_cores       # output shard rows per core
        self.HK = H // P              # contraction chunks over H
        self.TJ = T // P              # token chunks
        self.NI = I // P              # I tiles
        self.CT = C // P              # packed-slot tiles
        self.ISK = self.IS // P
        self.NQ = 4                   # down-proj H chunks
        self.HQ = H // self.NQ
        assert C % P == 0 and T % P == 0 and H % P == 0 and I % P == 0
        assert self.IS % P == 0 and self.TJ % 2 == 0


def build(cfg: Cfg, rs: bool = True, reps: int = 1):
    T, H, I, C = cfg.T, cfg.H, cfg.I, cfg.C
    HK, TJ, NI, CT, ISK = cfg.HK, cfg.TJ, cfg.NI, cfg.CT, cfg.ISK
    NQ, HQ = cfg.NQ, cfg.HQ
    TH = T // 2
    TJH = TJ // 2
    BIGC = 1.0e5

    nc = bacc.Bacc("TRN2", target_bir_lowering=False, debug=False,
                   num_devices=cfg.n_cores)

    xTt_d = nc.dram_tensor("xTt", [P, TJ * HK * P], dt.float32,
                           kind="ExternalInput").ap()
    xbf_d = nc.dram_tensor("xbf", [T, H], dt.bfloat16,
                           kind="ExternalInput").ap()
    rwT_d = nc.dram_tensor("rwT", [H, 8], dt.float32,
                           kind="ExternalInput").ap()
    wgu_d = nc.dram_tensor("wgu", [P, NI * 2 * HK * P], dt.bfloat16,
                           kind="ExternalInput").ap()
    wd_d = nc.dram_tensor("wd", [P, NQ * NI * HQ], dt.bfloat16,
                          kind="ExternalInput").ap()
    wgs_d = nc.dram_tensor("wgs", [P, ISK * HK * P], dt.bfloat16,
                           kind="ExternalInput").ap()
    wus_d = nc.dram_tensor("wus", [P, ISK * HK * P], dt.bfloat16,
                           kind="ExternalInput").ap()
    wds_d = nc.dram_tensor("wds", [P, ISK * H], dt.bfloat16,
                           kind="ExternalInput").ap()
    y_d = nc.dram_tensor("y", [cfg.TSH, H], dt.float32,
                         kind="ExternalOutput").ap()

    with tile.TileContext(nc) as tc:
        with tc.tile_pool(name="dram", bufs=1, space="DRAM") as dram:
            part = dram.tile([T, H], dt.float32)
            rs_out = dram.tile([cfg.TSH, H], dt.float32)
            for _rep in range(reps):
                _emit(nc, tc, cfg, part, rs_out, rs,
                      xTt_d, xbf_d, rwT_d, wgu_d, wd_d, wgs_d, wus_d, wds_d,
                      y_d)

    nc.compile()
    return nc


def _emit(nc, tc, cfg, part, rs_out, rs,
          xTt_d, xbf_d, rwT_d, wgu_d, wd_d, wgs_d, wus_d, wds_d, y_d):
    T, H, I, C = cfg.T, cfg.H, cfg.I, cfg.C
    HK, TJ, NI, CT, ISK = cfg.HK, cfg.TJ, cfg.NI, cfg.CT, cfg.ISK
    NQ, HQ = cfg.NQ, cfg.HQ
    TH = T // 2
    TJH = TJ // 2
    BIGC = 1.0e5
    WDQ = NI * (H // NQ)  # per-q chunk free elems of wd

    with tc.tile_pool(name="const", bufs=1) as const, \
         tc.tile_pool(name="keep", bufs=1) as keep, \
         tc.tile_pool(name="sb", bufs=3) as sb, \
         tc.tile_pool(name="pps", bufs=2, space="PSUM") as pps, \
         tc.tile_pool(name="pbig", bufs=4, space="PSUM") as pbig:

        # ---------------- constants ----------------
        iota_col_i = const.tile([P, P], dt.int32)
        nc.gpsimd.iota(iota_col_i[:], pattern=[[1, P]], base=0,
                       channel_multiplier=0)
        iota_row_i = const.tile([P, P], dt.int32)
        nc.gpsimd.iota(iota_row_i[:], pattern=[[0, P]], base=0,
                       channel_multiplier=1)
        iota_col_f = const.tile([P, P], dt.float32)
        nc.vector.tensor_copy(iota_col_f[:], iota_col_i[:])
        iota_row_f = const.tile([P, P], dt.float32)
        nc.vector.tensor_copy(iota_row_f[:], iota_row_i[:])
        ltri = const.tile([P, P], dt.bfloat16)  # ltri[k,m] = 1 if k<m
        nc.vector.tensor_tensor(out=ltri[:], in0=iota_row_f[:],
                                in1=iota_col_f[:], op=OP.is_lt)
        eye_f = const.tile([P, P], dt.float32)
        nc.vector.tensor_tensor(out=eye_f[:], in0=iota_row_f[:],
                                in1=iota_col_f[:], op=OP.is_equal)

        iotaC_i = const.tile([P, C], dt.int32)
        nc.gpsimd.iota(iotaC_i[:], pattern=[[1, C]], base=0,
                       channel_multiplier=0)
        iotaC_f = const.tile([P, C], dt.float32)
        nc.vector.tensor_copy(iotaC_f[:], iotaC_i[:])

        # empty packed slots get an out-of-bounds row (>= T)
        trash_i = const.tile([P, CT], dt.int32)
        nc.gpsimd.iota(trash_i[:], pattern=[[P, CT]], base=T,
                       channel_multiplier=1)
        trash_f = const.tile([P, CT], dt.float32)
        nc.vector.tensor_copy(trash_f[:], trash_i[:])

        # tokone rhs [P, TJ, 4]: col0 = t_lo, col1 = t_hi, col2 = 1,
        # col3 = routing score (filled after P2)
        lo_i = const.tile([P, TJ], dt.int32)
        nc.gpsimd.iota(lo_i[:], pattern=[[0, TJ]], base=0,
                       channel_multiplier=1)
        hi_i = const.tile([P, TJ], dt.int32)
        nc.gpsimd.iota(hi_i[:], pattern=[[1, TJ]], base=0,
                       channel_multiplier=0)
        tokone = keep.tile([P, TJ, 4], dt.bfloat16)
        nc.vector.tensor_copy(tokone[:, :, 0], lo_i[:])
        nc.vector.tensor_copy(tokone[:, :, 1], hi_i[:])
        nc.vector.memset(tokone[:, :, 2], 1.0)

        ones_col_bf = const.tile([P, 1], dt.bfloat16)
        nc.vector.memset(ones_col_bf[:], 1.0)
        ones_row_bf = const.tile([1, P], dt.bfloat16)
        nc.vector.memset(ones_row_bf[:], 1.0)

        # rolled router weights [P, HK, 8] fp32
        rw_sb = keep.tile([P, HK, 8], dt.float32)
        nc.sync.dma_start(rw_sb[:],
                          rwT_d.rearrange("(hk p) e -> p hk e", p=P))

        # xhat outlives the shared-MLP pools (used through P8): open first
        # so pool release order stays LIFO.
        xhat_cm = tc.tile_pool(name="xhatp", bufs=1)
        xhatp = xhat_cm.__enter__()

        # shared-MLP down weights + shared activations: alive through P10
        shp_cm = tc.tile_pool(name="shp", bufs=1)
        shp = shp_cm.__enter__()
        wds_sb = shp.tile([P, ISK * H], dt.bfloat16, tag="wds")
        nc.sync.dma_start(wds_sb[:], wds_d[:])

        logits = keep.tile([P, TJ, 8], dt.float32)
        act_sT = shp.tile([P, ISK * T], dt.bfloat16, tag="acts")

        # ==== P1: fp32 router + shared gate/up, by token half ====
        with tc.tile_pool(name="wgup", bufs=1) as wgup, \
             tc.tile_pool(name="xtbf_p", bufs=1) as xtbf_pool, \
             tc.tile_pool(name="p1s", bufs=3) as p1s, \
             tc.tile_pool(name="ppr", bufs=2, space="PSUM") as ppr:
            wgs_sb = wgup.tile([P, ISK * HK * P], dt.bfloat16, tag="wg")
            nc.sync.dma_start(wgs_sb[:], wgs_d[:])
            wus_sb = wgup.tile([P, ISK * HK * P], dt.bfloat16, tag="wu")
            nc.sync.dma_start(wus_sb[:], wus_d[:])
            for th in range(2):
                xtbf = xtbf_pool.tile([P, HK, TH], dt.bfloat16, tag="xtbf")
                for tjl in range(TJH):
                    tj = th * TJH + tjl
                    xcol = p1s.tile([P, HK, P], dt.float32, tag="stg_f")
                    nc.sync.dma_start(
                        xcol[:].rearrange("p hk t -> p (hk t)"),
                        xTt_d[:, tj * HK * P:(tj + 1) * HK * P])
                    nc.scalar.activation(
                        xtbf[:, :, tjl * P:(tjl + 1) * P], xcol[:],
                        AF.Copy)
                    pl = ppr.tile([P, 8], dt.float32, tag="plog")
                    for hk in range(HK):
                        nc.tensor.matmul(
                            pl[:], xcol[:, hk, :], rw_sb[:, hk, :],
                            start=(hk == 0), stop=(hk == HK - 1))
                    nc.vector.tensor_copy(logits[:, tj, :], pl[:])

                # shared gate/up on this token half
                for isx in range(ISK):
                    for tn in range(TH // 512):
                        t0 = tn * 512
                        pg = pbig.tile([P, 512], dt.float32, tag="pbig")
                        pu = pbig.tile([P, 512], dt.float32, tag="pbig")
                        for hk in range(HK):
                            nc.tensor.matmul(
                                pg[:],
                                wgs_sb[:, (isx * HK + hk) * P:
                                       (isx * HK + hk + 1) * P],
                                xtbf[:, hk, t0:t0 + 512],
                                start=(hk == 0), stop=(hk == HK - 1))
                        for hk in range(HK):
                            nc.tensor.matmul(
                                pu[:],
                                wus_sb[:, (isx * HK + hk) * P:
                                       (isx * HK + hk + 1) * P],
                                xtbf[:, hk, t0:t0 + 512],
                                start=(hk == 0), stop=(hk == HK - 1))
                        sil = sb.tile([P, 512], dt.float32, tag="sil")
                        nc.scalar.activation(sil[:], pg[:], AF.Silu)
                        o0 = isx * T + th * TH + t0
                        nc.vector.tensor_tensor(
                            out=act_sT[:, o0:o0 + 512],
                            in0=sil[:], in1=pu[:], op=OP.mult)

        # ============ P2: top-1 mask + sigmoid score ============
        maxv = keep.tile([P, TJ], dt.float32)
        for tj in range(TJ):
            m8 = sb.tile([P, 8], dt.float32, tag="m8")
            nc.vector.max(m8[:], logits[:, tj, :])
            nc.vector.tensor_copy(maxv[:, tj:tj + 1], m8[:, 0:1])
        sig = keep.tile([P, TJ], dt.float32)
        nc.scalar.activation(sig[:], maxv[:], AF.Sigmoid)
        mask = keep.tile([P, TJ], dt.float32)
        nc.vector.tensor_tensor(out=mask[:], in0=logits[:, :, 0],
                                in1=maxv[:], op=OP.is_equal)
        smine = keep.tile([P, TJ], dt.float32)
        nc.vector.tensor_tensor(out=smine[:], in0=mask[:], in1=sig[:],
                                op=OP.mult)
        nc.vector.tensor_copy(tokone[:, :, 3], smine[:])
        mask_bf = keep.tile([P, TJ], dt.bfloat16)
        nc.vector.tensor_copy(mask_bf[:], mask[:])

        # ============ P3: packed positions (prefix sums) ============
        pos_ps = pps.tile([P, TJ], dt.float32, bufs=1, tag="pos")
        nc.tensor.matmul(pos_ps[:], ltri[:], mask_bf[:],
                         start=True, stop=True)
        tot_ps = pps.tile([1, TJ], dt.float32, bufs=1, tag="tb")
        nc.tensor.matmul(tot_ps[:], ones_col_bf[:], mask_bf[:],
                         start=True, stop=True)
        tot_bf = sb.tile([1, TJ], dt.bfloat16, tag="totb")
        nc.vector.tensor_copy(tot_bf[:], tot_ps[:])
        bc_ps = pps.tile([P, TJ], dt.float32, bufs=1, tag="tb")
        nc.tensor.matmul(bc_ps[:], ones_row_bf[:], tot_bf[:],
                         start=True, stop=True)
        # exclusive scan along the TJ axis of the broadcast totals
        exa = sb.tile([P, TJ], dt.float32, tag="scan")
        nc.vector.memset(exa[:, 0:1], 0.0)
        if TJ > 1:
            nc.vector.tensor_copy(exa[:, 1:], bc_ps[:, :TJ - 1])
        sh = 1
        while sh < TJ:
            exb = sb.tile([P, TJ], dt.float32, tag="scan")
            nc.vector.tensor_copy(exb[:, :sh], exa[:, :sh])
            nc.vector.tensor_tensor(out=exb[:, sh:], in0=exa[:, sh:],
                                    in1=exa[:, :TJ - sh], op=OP.add)
            exa = exb
            sh *= 2
        posg = keep.tile([P, TJ], dt.float32)
        nc.vector.tensor_tensor(out=posg[:], in0=exa[:], in1=pos_ps[:],
                                op=OP.add)
        nmsk = sb.tile([P, TJ], dt.float32, tag="scan")
        nc.vector.tensor_scalar(out=nmsk[:], in0=mask[:],
                                scalar1=-BIGC, scalar2=BIGC,
                                op0=OP.mult, op1=OP.add)
        posm = keep.tile([P, TJ], dt.float32)
        nc.vector.tensor_tensor(out=posm[:], in0=posg[:], in1=nmsk[:],
                                op=OP.add)

        # ============ P4: 0/1 selection matrix (for index build) ========
        selp_cm = tc.tile_pool(name="selp", bufs=1)
        selp = selp_cm.__enter__()
        S01b = selp.tile([P, TJ * C], dt.bfloat16, tag="s01b")
        for tj in range(TJ):
            s01 = sb.tile([P, C], dt.float32, tag="s01")
            nc.vector.tensor_tensor(
                out=s01[:],
                in0=posm[:, tj:tj + 1].to_broadcast([P, C]),
                in1=iotaC_f[:], op=OP.is_equal)
            nc.vector.tensor_copy(S01b[:, tj * C:(tj + 1) * C], s01[:])

        # ===== P7: per-slot token index + score  =====
        dest_i = keep.tile([P, CT], dt.int32)
        s_col = keep.tile([P, CT], dt.bfloat16)
        for sc in range(CT):
            pd = pps.tile([P, 4], dt.float32, bufs=1, tag="pos")
            for tj in range(TJ):
                nc.tensor.matmul(pd[:],
                                 S01b[:, tj * C + sc * P:tj * C + (sc + 1) * P],
                                 tokone[:, tj, :],
                                 start=(tj == 0), stop=(tj == TJ - 1))
            # dest = lo + 128*hi  if occupied else trash row
            t1 = sb.tile([P, 1], dt.float32, tag="dsmall")
            nc.vector.tensor_scalar(out=t1[:], in0=pd[:, 1:2],
                                    scalar1=float(P), scalar2=None,
                                    op0=OP.mult)
            t1b = sb.tile([P, 1], dt.float32, tag="dsmall")
            nc.vector.tensor_tensor(out=t1b[:], in0=t1[:], in1=pd[:, 0:1],
                                    op=OP.add)
            t2 = sb.tile([P, 1], dt.float32, tag="dsmall")
            nc.vector.tensor_scalar(out=t2[:], in0=pd[:, 2:3],
                                    scalar1=-1.0, scalar2=1.0,
                                    op0=OP.mult, op1=OP.add)
            t3 = sb.tile([P, 1], dt.float32, tag="dsmall")
            nc.vector.tensor_tensor(out=t3[:], in0=t2[:],
                                    in1=trash_f[:, sc:sc + 1], op=OP.mult)
            t4 = sb.tile([P, 1], dt.float32, tag="dsmall")
            nc.vector.tensor_tensor(out=t4[:], in0=t3[:], in1=t1b[:],
                                    op=OP.add)
            nc.vector.tensor_copy(dest_i[:, sc:sc + 1], t4[:])
            nc.vector.tensor_copy(s_col[:, sc:sc + 1], pd[:, 3:4])

        selp_cm.__exit__(None, None, None)

        # ===== P6: gather routed tokens, scale by score, transpose =====
        xhat = xhatp.tile([P, HK, C], dt.bfloat16, tag="xhat")
        with tc.tile_pool(name="gatp", bufs=1) as gatp:
            xg = gatp.tile([P, CT * H], dt.bfloat16, tag="xg")
            nc.vector.memset(xg[:], 0.0)
            xgs = gatp.tile([P, CT * H], dt.bfloat16, tag="xgs")
            for ct in range(CT):
                nc.gpsimd.indirect_dma_start(
                    out=xg[:, ct * H:(ct + 1) * H],
                    out_offset=None,
                    in_=xbf_d[:],
                    in_offset=bass.IndirectOffsetOnAxis(
                        ap=dest_i[:, ct:ct + 1], axis=0),
                    bounds_check=T - 1,
                    oob_is_err=False)
                nc.vector.tensor_tensor(
                    out=xgs[:, ct * H:(ct + 1) * H],
                    in0=xg[:, ct * H:(ct + 1) * H],
                    in1=s_col[:, ct:ct + 1].to_broadcast([P, H]),
                    op=OP.mult)
                nc.sync.dma_start_transpose(
                    xhat[:, :, ct * P:(ct + 1) * P],
                    xgs[:, ct * H:(ct + 1) * H])

        # ============ P10: shared down-proj -> part[t, :] ============
        for tt in range(TJ):
            for hn in range(H // 512):
                psd = pbig.tile([P, 512], dt.float32, tag="pbig")
                for ik in range(ISK):
                    nc.tensor.matmul(
                        psd[:],
                        act_sT[:, ik * T + tt * P:ik * T + (tt + 1) * P],
                        wds_sb[:, ik * H + hn * 512:ik * H + (hn + 1) * 512],
                        start=(ik == 0), stop=(ik == ISK - 1))
                so = sb.tile([P, 512], dt.float32, tag="pout", bufs=6)
                nc.vector.tensor_copy(so[:], psd[:])
                # part writes go out on the Activation HWDGE queue so they
                # don't head-of-line block the weight streams on sync's queue
                nc.scalar.dma_start(
                    part[tt * P:(tt + 1) * P, hn * 512:(hn + 1) * 512],
                    so[:])
        shp_cm.__exit__(None, None, None)

        # ============ P8: expert gate_up^T then act^T ============
        with tc.tile_pool(name="apool", bufs=1) as apool:
            actT = apool.tile([P, NI * C], dt.bfloat16, tag="actT")
            with tc.tile_pool(name="wchp", bufs=3) as wchp:
                for ii in range(NI):
                    wch = wchp.tile([P, 2 * HK * P], dt.bfloat16, tag="wch")
                    nc.sync.dma_start(
                        wch[:],
                        wgu_d[:, ii * 2 * HK * P:(ii + 1) * 2 * HK * P])
                    pg = pbig.tile([P, C], dt.float32, tag="pbig")
                    pu = pbig.tile([P, C], dt.float32, tag="pbig")
                    for hk in range(HK):
                        nc.tensor.matmul(pg[:], wch[:, hk * P:(hk + 1) * P],
                                         xhat[:, hk, :],
                                         start=(hk == 0), stop=(hk == HK - 1))
                    for hk in range(HK):
                        nc.tensor.matmul(
                            pu[:], wch[:, (HK + hk) * P:(HK + hk + 1) * P],
                            xhat[:, hk, :],
                            start=(hk == 0), stop=(hk == HK - 1))
                    sil = sb.tile([P, C], dt.float32, tag="s01")
                    nc.scalar.activation(sil[:], pg[:], AF.Silu)
                    nc.vector.tensor_tensor(
                        out=actT[:, ii * C:(ii + 1) * C],
                        in0=sil[:], in1=pu[:], op=OP.mult)

            # ==== P9: expert down-proj -> packed rows, scatter-add ====
            with tc.tile_pool(name="rpool", bufs=1) as rpool, \
                 tc.tile_pool(name="wdp", bufs=2) as wdp:
                routed_sb = rpool.tile([P, CT * H], dt.float32, tag="routed")
                for q in range(NQ):
                    wdc = wdp.tile([P, NI * HQ], dt.bfloat16, tag="wdc")
                    nc.sync.dma_start(
                        wdc[:], wd_d[:, q * NI * HQ:(q + 1) * NI * HQ])
                    for ct in range(CT):
                        pdn = pbig.tile([P, HQ], dt.float32, tag="pbig")
                        for ik in range(NI):
                            nc.tensor.matmul(
                                pdn[:],
                                actT[:, ik * C + ct * P:ik * C + (ct + 1) * P],
                                wdc[:, ik * HQ:(ik + 1) * HQ],
                                start=(ik == 0), stop=(ik == NI - 1))
                        o0 = ct * H + q * HQ
                        if (q + ct) % 2 == 0:
                            nc.vector.tensor_copy(
                                routed_sb[:, o0:o0 + HQ], pdn[:])
                        else:
                            nc.scalar.activation(
                                routed_sb[:, o0:o0 + HQ], pdn[:], AF.Copy)

                # scatter-add packed rows into part
                for ct in range(CT):
                    nc.gpsimd.indirect_dma_start(
                        out=part[:],
                        out_offset=bass.IndirectOffsetOnAxis(
                            ap=dest_i[:, ct:ct + 1], axis=0),
                        in_=routed_sb[:, ct * H:(ct + 1) * H],
                        in_offset=None,
                        bounds_check=T - 1,
                        oob_is_err=False,
                        compute_op=OP.add)
        xhat_cm.__exit__(None, None, None)

        # ============ P12: ReduceScatter + output ============
        if rs:
            nc.gpsimd.collective_compute(
                "ReduceScatter", OP.add,
                replica_groups=[list(range(cfg.n_cores))],
                ins=[part.opt()],
                outs=[rs_out.opt()])
            src = rs_out
        else:
            src = part
        with tc.tile_pool(name="ooutp", bufs=2) as ooutp:
            for b0 in range(0, cfg.TSH, P):
                ot = ooutp.tile([P, H], dt.float32, tag="oout")
                nc.scalar.dma_start(ot[:], src[b0:b0 + P, :])
                nc.scalar.dma_start(y_d[b0:b0 + P, :], ot[:])


# dims of the real problem
CFG = Cfg(n_cores=8, T=2048, H=2048, I=4096, C=384)
_NC_CACHE = {}


def _get_nc(cfg, rs=True, reps=1):
    key = (cfg.n_cores, cfg.T, cfg.H, cfg.I, cfg.C, rs, reps)
    if key not in _NC_CACHE:
        _NC_CACHE[key] = build(cfg, rs=rs, reps=reps)
    return _NC_CACHE[key]


def make_in_maps(cfg, hidden_states, router_w, gate_up_proj, down_proj,
                 shared_gate_w, shared_up_w, shared_down_w):
    T, H, I, IS = cfg.T, cfg.H, cfg.I, cfg.IS
    HK, TJ, NI, ISK = cfg.HK, cfg.TJ, cfg.NI, cfg.ISK
    NQ, HQ = cfg.NQ, cfg.HQ
    x = np.ascontiguousarray(
        np.asarray(hidden_states, dtype=np.float32).reshape(T, H))
    # [p, tj, hk, t] = x[tj*128+t, hk*128+p]
    xTt = np.ascontiguousarray(
        x.reshape(TJ, P, HK, P).transpose(3, 0, 2, 1)).reshape(P, -1)
    xbf = np.ascontiguousarray(x.astype(BF16))
    router_w = np.asarray(router_w, dtype=np.float32)
    in_maps = []
    for c in range(cfg.n_cores):
        rw_roll = np.roll(router_w, -c, axis=0)  # row j = expert (c+j)%8
        gup = np.asarray(gate_up_proj[c], dtype=np.float32)
        g = gup[:, :I].reshape(HK, P, NI, P).transpose(1, 2, 0, 3)
        u = gup[:, I:].reshape(HK, P, NI, P).transpose(1, 2, 0, 3)
        wgu_t = np.ascontiguousarray(
            np.stack([g, u], axis=2).astype(BF16)).reshape(P, -1)
        wd = np.asarray(down_proj[c], dtype=np.float32)
        wd_t = np.ascontiguousarray(
            wd.reshape(NI, P, NQ, HQ).transpose(1, 2, 0, 3).astype(
                BF16)).reshape(P, -1)
        wgs = np.asarray(shared_gate_w[:, c * IS:(c + 1) * IS],
                         dtype=np.float32)
        wgs_t = np.ascontiguousarray(
            wgs.reshape(HK, P, ISK, P).transpose(1, 2, 0, 3).astype(
                BF16)).reshape(P, -1)
        wus = np.asarray(shared_up_w[:, c * IS:(c + 1) * IS],
                         dtype=np.float32)
        wus_t = np.ascontiguousarray(
            wus.reshape(HK, P, ISK, P).transpose(1, 2, 0, 3).astype(
                BF16)).reshape(P, -1)
        wds = np.asarray(shared_down_w[c * IS:(c + 1) * IS, :],
                         dtype=np.float32)
        wds_t = np.ascontiguousarray(
            wds.reshape(ISK, P, H).transpose(1, 0, 2).astype(
                BF16)).reshape(P, -1)
        in_maps.append({
            "xTt": xTt,
            "xbf": xbf,
            "rwT": np.ascontiguousarray(rw_roll.T),
            "wgu": wgu_t,
            "wd": wd_t,
            "wgs": wgs_t,
            "wus": wus_t,
            "wds": wds_t,
        })
    return in_maps


def kernel(hidden_states, router_w, gate_up_proj, down_proj,
           shared_gate_w, shared_up_w, shared_down_w):
    cfg = CFG
    orig_shape = np.asarray(hidden_states).shape
    nc = _get_nc(cfg)
    in_maps = make_in_maps(cfg, hidden_states, router_w, gate_up_proj,
                           down_proj, shared_gate_w, shared_up_w,
                           shared_down_w)
    res = run_bass_kernel_spmd(nc, in_maps, core_ids=list(range(cfg.n_cores)))
    y = np.concatenate([res.results[c]["y"] for c in range(cfg.n_cores)],
                       axis=0)
    return y.reshape(orig_shape).astype(np.float32)
